# revision 1
# baseline (speedup 1.0000x reference)
"""AttentionWithRotary on 8 trn2 NeuronCores.

Sharding: B*T = 4 frames; 8 cores = 4 frames x 2 query-halves (data
parallel on the frame axis, sequence-split within a frame pair).  Each
core computes the full qkv projection + LN + rotary for its frame (k/v
must cover all 1024 keys), then attention + output projection for its
512-row query half only.  No cross-core communication is needed.
Host side only shards/stacks inputs and re-assembles the output.
"""

import numpy as np
from functools import partial

import jax
import jax.numpy as jnp

jax.config.update("jax_default_matmul_precision", "highest")

DIM = 384
HEADS = 8
DH = DIM // HEADS
SCALE = DH ** -0.5
EPS = 1e-5
B, T, L = 1, 4, 1024
NC = 8
HALF = L // 2


def _ln(x, g, b):
    m = jnp.mean(x, axis=-1, keepdims=True)
    v = jnp.var(x, axis=-1, keepdims=True)
    return (x - m) * jax.lax.rsqrt(v + EPS) * g + b


def _rot_half(x):
    h = x.shape[-1] // 2
    return jnp.concatenate([-x[..., h:], x[..., :h]], axis=-1)


def _rotary_tables():
    inv_freq = 1.0 / (10000.0 ** (np.arange(0, DH, 2, dtype=np.float32) / DH))
    t = np.arange(L, dtype=np.float32)
    freqs = np.outer(t, inv_freq)
    emb = np.concatenate([freqs, freqs], axis=-1)
    return np.cos(emb).astype(np.float32), np.sin(emb).astype(np.float32)


@partial(jax.pmap, axis_name="c")
def _core(x_f, mask, q0, W_qkv, W_out, b_out, g_qkv, b_qkv, g_q, b_q,
          g_k, b_k, cos, sin):
    # x_f: [L, D] full frame for this core; q0: start row of query half.
    qkv = _ln(x_f, g_qkv, b_qkv) @ W_qkv                  # [L, 3D]
    q, k, v = jnp.split(qkv, 3, axis=-1)
    q = _ln(q, g_q, b_q)
    k = _ln(k, g_k, b_k)
    q = q.reshape(L, HEADS, DH)
    k = k.reshape(L, HEADS, DH)
    q = q * cos[:, None, :] + _rot_half(q) * sin[:, None, :]
    k = k * cos[:, None, :] + _rot_half(k) * sin[:, None, :]
    v = v.reshape(L, HEADS, DH)
    q = jax.lax.dynamic_slice_in_dim(q, q0, HALF, axis=0)  # [HALF, H, DH]
    aw = jnp.einsum("lhd,shd->hls", q, k) * SCALE          # [H, HALF, L]
    aw = jnp.where(mask[None, None, :] == 0, -jnp.inf, aw)
    p = jax.nn.softmax(aw, axis=-1)
    o = jnp.einsum("hls,shd->lhd", p, v).reshape(HALF, DIM)
    return o @ W_out.T + b_out                             # [HALF, D]


def kernel(x, attention_mask, W_qkv, W_out, b_out, g_qkv, b_qkv,
           g_q, b_q, g_k, b_k):
    x = np.asarray(x, dtype=np.float32)
    frames = x.reshape(B * T, L, DIM)
    cos, sin = _rotary_tables()
    mask = np.asarray(attention_mask, dtype=np.int32).reshape(L)

    # Per-core stacked inputs: core c -> frame c//2, query half c%2.
    xs = np.stack([frames[c // 2] for c in range(NC)])            # [8, L, D]
    q0s = np.array([(c % 2) * HALF for c in range(NC)], np.int32)  # [8]

    def rep(a):
        a = np.asarray(a, dtype=np.float32)
        return np.broadcast_to(a, (NC,) + a.shape).copy()

    out = _core(xs, rep(mask).astype(np.int32), q0s,
                rep(W_qkv), rep(W_out), rep(b_out), rep(g_qkv), rep(b_qkv),
                rep(g_q), rep(b_q), rep(g_k), rep(b_k), rep(cos), rep(sin))
    out = np.asarray(out)                                          # [8, HALF, D]
    return out.reshape(B, T, L, DIM).astype(np.float32)


# revision 2
# speedup vs baseline: 2.3141x; 2.3141x over previous
"""AttentionWithRotary on 8 trn2 NeuronCores.

Sharding: B*T = 4 frames; 8 cores = 4 frames x 2 query-halves (data
parallel on the frame axis, sequence-split within a frame pair).  Each
core receives only its unique 512-row half-frame; the full 1024-row
frame (needed for k/v) is reconstructed on-device by an all-gather
within each frame pair over NeuronLink.  Each core then computes the
qkv projection + LN + rotary for the frame and attention + output
projection for its query half.  Weights are cached on-device across
calls; rotary cos/sin tables are traced constants baked into the NEFF.
"""

import numpy as np
from functools import partial

import jax
import jax.numpy as jnp

jax.config.update("jax_default_matmul_precision", "highest")

DIM = 384
HEADS = 8
DH = DIM // HEADS
SCALE = DH ** -0.5
EPS = 1e-5
B, T, L = 1, 4, 1024
NC = 8
HALF = L // 2

PAIRS = [[0, 1], [2, 3], [4, 5], [6, 7]]


def _ln(x, g, b):
    m = jnp.mean(x, axis=-1, keepdims=True)
    v = jnp.var(x, axis=-1, keepdims=True)
    return (x - m) * jax.lax.rsqrt(v + EPS) * g + b


def _rot_half(x):
    h = x.shape[-1] // 2
    return jnp.concatenate([-x[..., h:], x[..., :h]], axis=-1)


def _rotary_tables():
    inv_freq = 1.0 / (10000.0 ** (np.arange(0, DH, 2, dtype=np.float32) / DH))
    t = np.arange(L, dtype=np.float32)
    freqs = np.outer(t, inv_freq)
    emb = np.concatenate([freqs, freqs], axis=-1)
    return np.cos(emb).astype(np.float32), np.sin(emb).astype(np.float32)


_COS, _SIN = _rotary_tables()


@partial(jax.pmap, axis_name="c")
def _core(x_h, mask, q0, W_qkv, W_out, b_out, g_qkv, b_qkv, g_q, b_q,
          g_k, b_k):
    # x_h: [HALF, D] this core's half-frame; gather the frame pair.
    x_f = jax.lax.all_gather(x_h, "c", axis_index_groups=PAIRS)
    x_f = x_f.reshape(L, DIM)
    cos = jnp.asarray(_COS)
    sin = jnp.asarray(_SIN)
    qkv = _ln(x_f, g_qkv, b_qkv) @ W_qkv                  # [L, 3D]
    q, k, v = jnp.split(qkv, 3, axis=-1)
    q = _ln(q, g_q, b_q)
    k = _ln(k, g_k, b_k)
    q = q.reshape(L, HEADS, DH)
    k = k.reshape(L, HEADS, DH)
    q = q * cos[:, None, :] + _rot_half(q) * sin[:, None, :]
    k = k * cos[:, None, :] + _rot_half(k) * sin[:, None, :]
    v = v.reshape(L, HEADS, DH)
    q = jax.lax.dynamic_slice_in_dim(q, q0, HALF, axis=0)  # [HALF, H, DH]
    aw = jnp.einsum("lhd,shd->hls", q, k) * SCALE          # [H, HALF, L]
    aw = jnp.where(mask[None, None, :] == 0, -jnp.inf, aw)
    p = jax.nn.softmax(aw, axis=-1)
    o = jnp.einsum("hls,shd->lhd", p, v).reshape(HALF, DIM)
    return o @ W_out.T + b_out                             # [HALF, D]


_Q0S = np.array([(c % 2) * HALF for c in range(NC)], np.int32)
_weight_cache = {}


def _rep_dev(name, a):
    """Replicate a small array to all 8 devices, cached across calls."""
    a = np.ascontiguousarray(np.asarray(a, dtype=np.float32))
    key = (name, a.shape, hash(a.tobytes()))
    hit = _weight_cache.get(name)
    if hit is not None and hit[0] == key:
        return hit[1]
    stacked = np.broadcast_to(a, (NC,) + a.shape)
    dev = jax.device_put_sharded([np.asarray(s) for s in stacked],
                                 jax.devices()[:NC])
    _weight_cache[name] = (key, dev)
    return dev


def kernel(x, attention_mask, W_qkv, W_out, b_out, g_qkv, b_qkv,
           g_q, b_q, g_k, b_k):
    x = np.asarray(x, dtype=np.float32)
    halves = x.reshape(NC, HALF, DIM)          # core c -> rows of frame c//2
    mask = np.asarray(attention_mask, dtype=np.int32).reshape(L)
    mask_rep = np.broadcast_to(mask, (NC, L)).copy()

    out = _core(halves, mask_rep, _Q0S,
                _rep_dev("W_qkv", W_qkv), _rep_dev("W_out", W_out),
                _rep_dev("b_out", b_out), _rep_dev("g_qkv", g_qkv),
                _rep_dev("b_qkv", b_qkv), _rep_dev("g_q", g_q),
                _rep_dev("b_q", b_q), _rep_dev("g_k", g_k),
                _rep_dev("b_k", b_k))
    out = np.asarray(out)                      # [8, HALF, D]
    return out.reshape(B, T, L, DIM).astype(np.float32)


# revision 3
# speedup vs baseline: 2.7346x; 1.1817x over previous
"""AttentionWithRotary on 8 trn2 NeuronCores.

Sharding: B*T = 4 frames; 8 cores = 4 frames x 2 query-halves (data
parallel on the frame axis, sequence-split within a frame pair).  Each
core receives only its unique 512-row half-frame; the full 1024-row
frame (needed for k/v) is reconstructed on-device by an all-gather
within each frame pair over NeuronLink.  Each core then computes the
qkv projection + LN + rotary for the frame and attention + output
projection for its query half.  Weights are cached on-device across
calls; rotary cos/sin tables are traced constants baked into the NEFF.
"""

import numpy as np
from functools import partial

import jax
import jax.numpy as jnp

jax.config.update("jax_default_matmul_precision", "highest")

DIM = 384
HEADS = 8
DH = DIM // HEADS
SCALE = DH ** -0.5
EPS = 1e-5
B, T, L = 1, 4, 1024
NC = 8
HALF = L // 2

PAIRS = [[0, 1], [2, 3], [4, 5], [6, 7]]


def _ln(x, g, b):
    m = jnp.mean(x, axis=-1, keepdims=True)
    v = jnp.var(x, axis=-1, keepdims=True)
    return (x - m) * jax.lax.rsqrt(v + EPS) * g + b


def _rot_half(x):
    h = x.shape[-1] // 2
    return jnp.concatenate([-x[..., h:], x[..., :h]], axis=-1)


def _rotary_tables():
    inv_freq = 1.0 / (10000.0 ** (np.arange(0, DH, 2, dtype=np.float32) / DH))
    t = np.arange(L, dtype=np.float32)
    freqs = np.outer(t, inv_freq)
    emb = np.concatenate([freqs, freqs], axis=-1)
    return np.cos(emb).astype(np.float32), np.sin(emb).astype(np.float32)


_COS, _SIN = _rotary_tables()


@partial(jax.pmap, axis_name="c")
def _core(x_h, mask, q0, W_qkv, W_out, b_out, g_qkv, b_qkv, g_q, b_q,
          g_k, b_k):
    # x_h: [HALF, D] this core's half-frame; gather the frame pair.
    x_f = jax.lax.all_gather(x_h, "c", axis_index_groups=PAIRS)
    x_f = x_f.reshape(L, DIM)
    cos = jnp.asarray(_COS)
    sin = jnp.asarray(_SIN)
    # k/v need the full frame; q only this core's own rows (= x_h).
    ln_f = _ln(x_f, g_qkv, b_qkv)
    kv = ln_f @ W_qkv[:, DIM:]                             # [L, 2D]
    k, v = jnp.split(kv, 2, axis=-1)
    q = jax.lax.dynamic_slice_in_dim(ln_f, q0, HALF, axis=0) @ W_qkv[:, :DIM]
    q = _ln(q, g_q, b_q)
    k = _ln(k, g_k, b_k)
    q = q.reshape(HALF, HEADS, DH)
    k = k.reshape(L, HEADS, DH)
    cos_q = jax.lax.dynamic_slice_in_dim(cos, q0, HALF, axis=0)
    sin_q = jax.lax.dynamic_slice_in_dim(sin, q0, HALF, axis=0)
    q = q * cos_q[:, None, :] + _rot_half(q) * sin_q[:, None, :]
    k = k * cos[:, None, :] + _rot_half(k) * sin[:, None, :]
    v = v.reshape(L, HEADS, DH)
    aw = jnp.einsum("lhd,shd->hls", q, k) * SCALE          # [H, HALF, L]
    aw = jnp.where(mask[None, None, :] == 0, -jnp.inf, aw)
    p = jax.nn.softmax(aw, axis=-1)
    o = jnp.einsum("hls,shd->lhd", p, v).reshape(HALF, DIM)
    return o @ W_out.T + b_out                             # [HALF, D]


_Q0S = np.array([(c % 2) * HALF for c in range(NC)], np.int32)
_weight_cache = {}


def _rep_dev(name, a):
    """Replicate a small array to all 8 devices, cached across calls."""
    a = np.ascontiguousarray(np.asarray(a, dtype=np.float32))
    key = (name, a.shape, hash(a.tobytes()))
    hit = _weight_cache.get(name)
    if hit is not None and hit[0] == key:
        return hit[1]
    stacked = np.broadcast_to(a, (NC,) + a.shape)
    dev = jax.device_put_sharded([np.asarray(s) for s in stacked],
                                 jax.devices()[:NC])
    _weight_cache[name] = (key, dev)
    return dev


def kernel(x, attention_mask, W_qkv, W_out, b_out, g_qkv, b_qkv,
           g_q, b_q, g_k, b_k):
    x = np.asarray(x, dtype=np.float32)
    halves = x.reshape(NC, HALF, DIM)          # core c -> rows of frame c//2
    mask = np.asarray(attention_mask, dtype=np.int32).reshape(L)
    mask_rep = np.broadcast_to(mask, (NC, L)).copy()

    out = _core(halves, mask_rep, _Q0S,
                _rep_dev("W_qkv", W_qkv), _rep_dev("W_out", W_out),
                _rep_dev("b_out", b_out), _rep_dev("g_qkv", g_qkv),
                _rep_dev("b_qkv", b_qkv), _rep_dev("g_q", g_q),
                _rep_dev("b_q", b_q), _rep_dev("g_k", g_k),
                _rep_dev("b_k", b_k))
    out = np.asarray(out)                      # [8, HALF, D]
    return out.reshape(B, T, L, DIM).astype(np.float32)
